# revision 1
# baseline (speedup 1.0000x reference)
"""MoE BatchedExperts kernel for 8 trn2 NeuronCores.

Strategy: expert parallelism with host-side top-k dispatch and exact load
balancing. Each token has TOP_K=2 nonzero routing weights; core c processes
a fixed per-core "slot structure" of expert token groups chosen so all
cores get ~N*K/E tokens (the hot experts are split across cores). All
matmuls run bf16 (1 row/cycle, same as fp32r, but half the DMA/SBUF and no
min-moving-dim constraint), PSUM accumulates fp32; measured end-to-end
rel err ~3e-3 vs the fp64 reference (gate 2e-2).

Per core, per group g (tokens gathered+transposed on host to xT [D, S_g]):
  h  = gelu(w0_g^T-tiles @ xT + b0)   [F-part, S_g]  tokens on moving dim
  yT = w1_g-tiles @ h                 [D-part, S_g]  tokens on moving dim
Host combines: out[idx] += r * yT.T rows; b1 folded in via routing @ b1.

Tokens stay on the PE moving dim in both phases so group sizes need no
128-padding. A few zero-filled warmup matmuls keep the PE busy (and ramp
its DVFS p-state) while the first input DMAs land.
"""

import numpy as np
import ml_dtypes

import concourse.bacc as bacc
import concourse.mybir as mybir
from concourse.tile import TileContext
from concourse.bass_utils import run_bass_kernel_spmd

F32 = mybir.dt.float32
BF16 = mybir.dt.bfloat16

N, D, E, F = 4096, 1024, 8, 2048
P = 128
KD = D // P            # 8  k-tiles for mm1 (contract D)
KF = F // P            # 16 k-tiles for mm2 (contract F)
DO = D // P            # 8  output d-tiles for mm2

_cache: dict[tuple, object] = {}


def _chunks_of(size: int) -> list[int]:
    """Split a group into near-equal moving-dim chunks <=512 (>=~250 keeps
    the per-matmul LDWEIGHTS (~97ns) hidden behind the previous matmul)."""
    n = -(-size // 512)
    base, rem = divmod(size, n)
    return [base + 1] * rem + [base] * (n - rem)


def build_program(sizes: tuple[int, ...]):
    """Bass program for one core: len(sizes) expert groups of fixed widths."""
    G = len(sizes)
    T = sum(sizes)
    goffs = [0, *np.cumsum(sizes).tolist()]
    chunks = [_chunks_of(s) for s in sizes]

    nc = bacc.Bacc("TRN2", target_bir_lowering=False, debug=False)
    xT = nc.dram_tensor("xT", [D, T], BF16, kind="ExternalInput")
    w0d = [nc.dram_tensor(f"w0_{g}", [D, F], BF16, kind="ExternalInput")
           for g in range(G)]
    w1d = [nc.dram_tensor(f"w1_{g}", [F, D], BF16, kind="ExternalInput")
           for g in range(G)]
    # b0 pre-arranged [128, G*KF] on the host (contiguous per-partition rows)
    b0 = nc.dram_tensor("b0", [P, G * KF], F32, kind="ExternalInput")
    yT = nc.dram_tensor("yT", [D, T], F32, kind="ExternalOutput")

    xT_r = xT.rearrange("(ko p) t -> p ko t", p=P)
    w0_r = [w.rearrange("(ko p) f -> p ko f", p=P) for w in w0d]
    w1_r = [w.rearrange("(ko p) d -> p ko d", p=P) for w in w1d]

    with TileContext(nc) as tc:
        with tc.tile_pool(name="const", bufs=1) as const, \
             tc.tile_pool(name="xpool", bufs=1) as xpool, \
             tc.tile_pool(name="hpool", bufs=1) as hpool, \
             tc.tile_pool(name="wpool", bufs=G + 1) as wpool, \
             tc.tile_pool(name="ypool", bufs=3) as ypool, \
             tc.tile_pool(name="psum", bufs=8, space="PSUM") as psum:

            # ---- PE warmup: zero matmuls bridge the first-input DMA
            # latency (~15us of engine startup + slow-window transfers)
            # and ramp the PE's DVFS p-state ----
            warm = const.tile([P, 512], BF16, name="warm")
            nc.vector.memset(warm[:], 0.0)
            for i in range(12):
                pw = psum.tile([P, 512], F32, tag="ps", name=f"warm{i}")
                nc.tensor.matmul(pw, warm[:, 0:P], warm[:], start=True,
                                 stop=True)

            # ---- input DMAs ----
            # Greedy issue turned out to beat every gated/serialized
            # variant: the DMA path runs well below nominal bandwidth for
            # the first ~15us, and strict ordering turns that slowness
            # into summed latencies while greedy overlap amortizes it.
            # scalar ring: x + b0 only — the gelu ACTs execute on the
            # scalar engine, so bulk-weight descriptor pumping there would
            # stall ACT drains and clog PSUM banks.
            x_sb = xpool.tile([P, KD, T], BF16, name="x")
            b0_sb = const.tile([P, G * KF], F32, name="b0")
            first = True
            for g in range(G):
                off = goffs[g]
                for c in chunks[g]:
                    nc.scalar.dma_start(x_sb[:, :, off:off + c],
                                        xT_r[:, :, off:off + c])
                    off += c
                    if first:
                        nc.scalar.dma_start(b0_sb[:], b0[:, :])
                        first = False

            # sync ring, submit order = priority order: w0 g0 front tiles
            # first, then the rest of w0, then w1 (phase 2 only), then y
            w0_sb, w1_sb = [], []
            for g in range(G):
                w = wpool.tile([P, KD, F], BF16, tag="wbig", name=f"w0_{g}")
                w0_sb.append(w)
                fo_slices = ([0, 256, 1024, 2048] if g == 0
                             else [0, 1024, 2048])
                for a, b in zip(fo_slices, fo_slices[1:]):
                    nc.sync.dma_start(w[:, :, a:b], w0_r[g][:, :, a:b])
            for g in reversed(range(G)):
                w = wpool.tile([P, KF, D], BF16, tag="wbig", name=f"w1_{g}")
                w1_sb.insert(0, w)
                for a in (0, 512):
                    nc.sync.dma_start(w[:, :, a:a + 512],
                                      w1_r[g][:, :, a:a + 512])

            # h = gelu(x @ w0 + b0), [F-part, T-free], groups concatenated
            h_sb = hpool.tile([P, KF, T], BF16, name="h")

            # ---- phase 1: mm1 + gelu ----
            for g in range(G):
                off = goffs[g]
                for c in chunks[g]:
                    for fo in range(KF):
                        ps = psum.tile([P, 512], F32, tag="ps",
                                       name=f"ps1_{g}_{off}_{fo}")[:, :c]
                        for k in range(KD):
                            nc.tensor.matmul(
                                ps, w0_sb[g][:, k, fo * P:(fo + 1) * P],
                                x_sb[:, k, off:off + c],
                                start=(k == 0), stop=(k == KD - 1))
                        nc.scalar.activation(
                            h_sb[:, fo, off:off + c], ps,
                            mybir.ActivationFunctionType.Gelu,
                            bias=b0_sb[:, g * KF + fo:g * KF + fo + 1])
                    off += c

            # ---- phase 2: mm2 ----
            # groups reversed: first group's h drained long ago, and the
            # final y store (inside the measured tail) is a small chunk
            for g in reversed(range(G)):
                off = goffs[g]
                for c in chunks[g]:
                    for do in range(DO):
                        ps2 = psum.tile([P, 512], F32, tag="ps",
                                        name=f"ps2_{g}_{off}_{do}")[:, :c]
                        for k in range(KF):
                            nc.tensor.matmul(
                                ps2, w1_sb[g][:, k, do * P:(do + 1) * P],
                                h_sb[:, k, off:off + c],
                                start=(k == 0), stop=(k == KF - 1))
                        y_sb = ypool.tile([P, 512], F32, tag="y",
                                          name=f"y_{g}_{off}_{do}")[:, :c]
                        nc.vector.tensor_copy(y_sb, ps2)
                        nc.sync.dma_start(
                            yT[do * P:(do + 1) * P, off:off + c], y_sb)
                    off += c

    nc.compile()
    return nc


def _plan(counts):
    """Choose per-core slot sizes (S1, S2) and assign expert token pieces.

    Minimizes T = S1 + S2 such that the 8 experts can be covered by 8
    pieces of size <= S1 plus 8 of size <= S2 (pieces of one expert may
    live on different cores). Falls back to one-slot-per-core (pure expert
    parallelism) if the search fails.
    """
    cmax = int(max(counts))
    order = sorted(range(E), key=lambda e: -counts[e])
    csort = [int(counts[e]) for e in order]

    def assign(S1, S2):
        from functools import lru_cache

        @lru_cache(maxsize=None)
        def feas(i, a, b):
            if i == len(csort):
                return ()
            c = csort[i]
            opts = []
            if c <= S1: opts.append((1, 0))
            if c <= S2: opts.append((0, 1))
            if c <= 2 * S2: opts.append((0, 2))
            if c <= S1 + S2: opts.append((1, 1))
            if c <= 2 * S1: opts.append((2, 0))
            if c <= S1 + 2 * S2: opts.append((1, 2))
            if c <= 2 * S1 + S2: opts.append((2, 1))
            opts.sort(key=lambda uv: (uv[0] + uv[1], S1 * uv[0] + S2 * uv[1]))
            for u, v in opts:
                if u <= a and v <= b:
                    rest = feas(i + 1, a - u, b - v)
                    if rest is not None:
                        return ((u, v),) + rest
            return None

        return feas(0, 8, 8)

    best = None
    for T in range(-(-N * 2 // E), cmax + 1):
        for S1 in range(-(-T // 2), T):
            S2 = T - S1
            sol = assign(S1, S2)
            if sol is not None:
                best = (S1, S2, sol)
                break
        if best:
            break
    if best is None:
        sizes = (cmax,)
        cores = [[(e, 0, int(counts[e]))] for e in range(E)]
        return sizes, cores

    S1, S2, sol = best
    s1_pieces, s2_pieces = [], []
    for i, (u, v) in enumerate(sol):
        e, c = order[i], csort[i]
        caps = [S1] * u + [S2] * v
        lo_ = 0
        for j, cap in enumerate(caps):
            take = min(cap, c - lo_)
            # ensure later pieces aren't left with more than they can hold
            take = max(take, c - lo_ - sum(caps[j + 1:]))
            (s1_pieces if cap == S1 else s2_pieces).append((e, lo_, take))
            lo_ += take
    while len(s1_pieces) < 8:
        s1_pieces.append((0, 0, 0))
    while len(s2_pieces) < 8:
        s2_pieces.append((0, 0, 0))
    sizes = (S1, S2)
    cores = [[s1_pieces[i], s2_pieces[i]] for i in range(8)]
    return sizes, cores


def kernel(x, routing_tensor, w0, b0, w1, b1):
    x = np.ascontiguousarray(np.asarray(x, dtype=np.float32))
    routing = np.asarray(routing_tensor, dtype=np.float32)
    w0 = np.asarray(w0, dtype=np.float32)
    b0 = np.asarray(b0, dtype=np.float32)
    w1 = np.asarray(w1, dtype=np.float32)
    b1 = np.asarray(b1, dtype=np.float32)

    idx = [np.nonzero(routing[:, e])[0] for e in range(E)]
    counts = [len(i) for i in idx]
    sizes, cores = _plan(counts)
    G = len(sizes)
    T = sum(sizes)
    goffs = np.concatenate([[0], np.cumsum(sizes)])

    nc = _cache.get(sizes)
    if nc is None:
        nc = _cache[sizes] = build_program(sizes)

    w0_bf = [np.ascontiguousarray(w0[e], dtype=ml_dtypes.bfloat16)
             for e in range(E)]
    w1_bf = [np.ascontiguousarray(w1[e], dtype=ml_dtypes.bfloat16)
             for e in range(E)]
    b0_cols = [np.ascontiguousarray(b0[e, 0].reshape(KF, P).T)
               for e in range(E)]

    in_maps = []
    for core in cores:
        xTc = np.zeros((D, T), dtype=ml_dtypes.bfloat16)
        b0c = np.empty((P, G * KF), dtype=np.float32)
        m = {"xT": xTc, "b0": b0c}
        for g, (e, lo, cnt) in enumerate(core):
            tok = idx[e][lo:lo + cnt]
            xTc[:, goffs[g]:goffs[g] + cnt] = \
                x[tok].T.astype(ml_dtypes.bfloat16)
            b0c[:, g * KF:(g + 1) * KF] = b0_cols[e]
            m[f"w0_{g}"] = w0_bf[e]
            m[f"w1_{g}"] = w1_bf[e]
        in_maps.append(m)

    res = run_bass_kernel_spmd(nc, in_maps, core_ids=list(range(8)))

    # combine: out = routing @ b1 + sum of r_e-scaled group outputs
    out = routing @ b1[:, 0, :]
    for ci, core in enumerate(cores):
        yT = res.results[ci]["yT"]
        for g, (e, lo, cnt) in enumerate(core):
            if cnt == 0:
                continue
            tok = idx[e][lo:lo + cnt]
            out[tok] += routing[tok, e:e + 1] * yT[:, goffs[g]:goffs[g] + cnt].T
    return out.astype(np.float32)



# revision 9
# speedup vs baseline: 1.0042x; 1.0042x over previous
"""MoE BatchedExperts kernel for 8 trn2 NeuronCores.

Strategy: expert parallelism with host-side top-k dispatch and exact load
balancing. Each token has TOP_K=2 nonzero routing weights; core c processes
a fixed per-core "slot structure" of expert token groups chosen so all
cores get ~N*K/E tokens (the hot experts are split across cores). All
matmuls run bf16 (1 row/cycle, same as fp32r, but half the DMA/SBUF and no
min-moving-dim constraint), PSUM accumulates fp32.

Per core, per group g (tokens gathered+transposed on host):
  h  = gelu(w0_g^T-tiles @ x + b0)    [F-part, S_g]  tokens on moving dim
  yT = w1_g-tiles @ h                 [D-part, S_g]  tokens on moving dim
Host combines: out[idx] += r * yT.T rows; b1 folded in via routing @ b1.

v2 notes (head/tail optimization, from NTFF trace analysis):
- All HBM inputs are host-permuted so every DMA is [128 partitions x
  contiguous bytes]: x per-chunk k-major [P, KD*c], weights k-major flat
  [P, KD*F] sliced along contiguous fo-blocks. 128 descriptors per DMA,
  2-16KB lines (vs 1024 sub-2KB descriptors before).
- Weight slices issue smallest-first on the sync HWDGE ring so mm1 can
  start as soon as ~0.8MB has landed; x rides the scalar HWDGE ring split
  into k-pieces; y stores ride the gpsimd SWDGE ring so they never queue
  behind weight loads.
- y is stored bf16 (halves store traffic; adds ~4e-4 rel err).
- Warmup matmuls accumulate into one PSUM bank (no pool-recycle stalls)
  to bridge the ~7us engine preamble + first-DMA latency and warm the
  PE HAM clock gate.
"""

import numpy as np
import ml_dtypes

import concourse.bacc as bacc
import concourse.mybir as mybir
from concourse.tile import TileContext
from concourse.bass_utils import run_bass_kernel_spmd

F32 = mybir.dt.float32
BF16 = mybir.dt.bfloat16

N, D, E, F = 4096, 1024, 8, 2048
P = 128
KD = D // P            # 8  k-tiles for mm1 (contract D)
KF = F // P            # 16 k-tiles for mm2 (contract F)
DO = D // P            # 8  output d-tiles for mm2
W0FLAT = KD * F        # 16384 flat bf16 elems/partition for one w0
W1FLAT = KF * D        # 16384 for one w1

# fo-blocks of w0 (by output-f column), smallest first so mm1 starts early
W0_SLICES = [(0, 128), (128, 512), (512, 1024), (1024, 2048)]
W1_SLICES = [(0, 512), (512, 1024)]

# swappable for CoreSim debugging (its interpreter lacks Gelu)
ACT_FN = mybir.ActivationFunctionType.Gelu

_cache: dict[tuple, object] = {}


def _chunks_of(size: int) -> list[int]:
    """Split a group into near-equal moving-dim chunks <=512 (>=~258 keeps
    the per-matmul LDWEIGHTS (~107ns) hidden behind the matmul)."""
    n = -(-size // 512)
    base, rem = divmod(size, n)
    return [base + 1] * rem + [base] * (n - rem)


def build_program(sizes: tuple[int, ...]):
    """Bass program for one core: len(sizes) expert groups of fixed widths."""
    G = len(sizes)
    chunks = [_chunks_of(s) for s in sizes]
    # flat chunk list [(g, chunk_idx, size)]
    flat = [(g, ci, c) for g in range(G) for ci, c in enumerate(chunks[g])]

    nc = bacc.Bacc("TRN2", target_bir_lowering=False, debug=False)
    xd = [nc.dram_tensor(f"x_{g}_{ci}", [P, KD * c], BF16,
                         kind="ExternalInput") for g, ci, c in flat]
    w0d = [nc.dram_tensor(f"w0_{g}", [P, W0FLAT], BF16, kind="ExternalInput")
           for g in range(G)]
    w1d = [nc.dram_tensor(f"w1_{g}", [P, W1FLAT], BF16, kind="ExternalInput")
           for g in range(G)]
    # b0 pre-arranged [128, G*KF] on the host (per-partition bias columns)
    b0 = nc.dram_tensor("b0", [P, G * KF], F32, kind="ExternalInput")
    # yT bf16, per (group, chunk): [P, DO*c] d-major flat
    yd = [nc.dram_tensor(f"y_{g}_{ci}", [P, DO * c], BF16,
                         kind="ExternalOutput") for g, ci, c in flat]

    with TileContext(nc) as tc:
        with tc.tile_pool(name="const", bufs=1) as const, \
             tc.tile_pool(name="xpool", bufs=1) as xpool, \
             tc.tile_pool(name="hpool", bufs=1) as hpool, \
             tc.tile_pool(name="wpool", bufs=G + 1) as wpool, \
             tc.tile_pool(name="ypool", bufs=4) as ypool, \
             tc.tile_pool(name="psum", bufs=8, space="PSUM") as psum:

            # ---- input DMAs: issue everything up front, priority order ----
            # scalar HWDGE ring: x (+b0). First chunk split by k-pairs so
            # the first matmuls can start before the whole chunk lands.
            # NOTE: pool slots are keyed by *tag* (not name) — per-chunk
            # tiles need distinct tags or they'd share one slot.
            x_sb = {}
            for i, (g, ci, c) in enumerate(flat):
                x_sb[(g, ci)] = xpool.tile([P, KD * c], BF16,
                                           tag=f"x{g}_{ci}", name=f"x{g}_{ci}")
            b0_sb = const.tile([P, G * KF], F32, tag="b0", name="b0")

            g0, c0i, c0 = flat[0]
            t0 = x_sb[(g0, c0i)]
            for k in range(0, KD, 2):
                nc.scalar.dma_start(t0[:, k * c0:(k + 2) * c0],
                                    xd[0][:, k * c0:(k + 2) * c0])
            nc.scalar.dma_start(b0_sb[:], b0[:, :])
            for i, (g, ci, c) in enumerate(flat):
                if i == 0:
                    continue
                nc.scalar.dma_start(x_sb[(g, ci)][:], xd[i][:])

            # sync HWDGE ring: weights. w0 g0 smallest slice first, then the
            # rest of w0, then w1 (phase 2) in reverse group order.
            # host ships w0 in the 4-slice block flat layout for every
            # group; later groups just use coarser DMAs over the same flat
            # (flat col ranges are layout-agnostic, only w0_ap decode isn't)
            w0_sb, w1_sb = [], []
            for g in range(G):
                w = wpool.tile([P, W0FLAT], BF16, tag="wbig", name=f"w0_{g}")
                w0_sb.append(w)
                cuts = ([0, KD * 128, KD * 512, KD * 1024, W0FLAT] if g == 0
                        else [0, W0FLAT // 2, W0FLAT])
                for a, b in zip(cuts, cuts[1:]):
                    nc.sync.dma_start(w[:, a:b], w0d[g][:, a:b])
            for g in reversed(range(G)):
                w = wpool.tile([P, W1FLAT], BF16, tag="wbig", name=f"w1_{g}")
                w1_sb.insert(0, w)
                for a, b in W1_SLICES:
                    nc.sync.dma_start(w[:, KF * a:KF * b],
                                      w1d[g][:, KF * a:KF * b])

            # ---- PE warmup: accumulating matmuls on one PSUM bank bridge
            # the engine preamble + first-DMA latency and ramp the HAM
            # clock gate. No pool recycling -> no drain stalls. ----
            warm = const.tile([P, 512], BF16, tag="warm", name="warm")
            nc.vector.memset(warm[:], 0.0)
            pw = psum.tile([P, 512], F32, tag="ps", name="warm")
            NWARM = 12
            for i in range(NWARM):
                nc.tensor.matmul(pw, warm[:, 0:P], warm[:],
                                 start=(i == 0), stop=(i == NWARM - 1))

            def w0_ap(g, k, fo):
                """[128,128] stationary AP for w0 block containing col fo."""
                for a, b in W0_SLICES:
                    if a <= fo * P < b:
                        off = KD * a + k * (b - a) + (fo * P - a)
                        return w0_sb[g][:, off:off + P]
                raise AssertionError

            def w1_ap(g, k, do):
                for a, b in W1_SLICES:
                    if a <= do * P < b:
                        off = KF * a + k * (b - a) + (do * P - a)
                        return w1_sb[g][:, off:off + P]
                raise AssertionError

            # h = gelu(x @ w0 + b0), [F-part, tokens] per (group, chunk)
            h_sb = {(g, ci): hpool.tile([P, KF * c], BF16,
                                        tag=f"h{g}_{ci}", name=f"h{g}_{ci}")
                    for g, ci, c in flat}

            # ---- phase 1: mm1 + gelu ----
            for g, ci, c in flat:
                xt = x_sb[(g, ci)]
                ht = h_sb[(g, ci)]
                for fo in range(KF):
                    ps = psum.tile([P, 512], F32, tag="ps",
                                   name=f"ps1_{g}_{ci}_{fo}")[:, :c]
                    for k in range(KD):
                        nc.tensor.matmul(ps, w0_ap(g, k, fo),
                                         xt[:, k * c:k * c + c],
                                         start=(k == 0), stop=(k == KD - 1))
                    nc.scalar.activation(
                        ht[:, fo * c:(fo + 1) * c], ps,
                        ACT_FN,
                        bias=b0_sb[:, g * KF + fo:g * KF + fo + 1])

            # ---- phase 2: mm2 ----
            # groups reversed: first group's h drained long ago, and the
            # final y store (inside the measured tail) comes from the
            # last, smaller chunk.
            for g, ci, c in reversed(flat):
                ht = h_sb[(g, ci)]
                yi = flat.index((g, ci, c))
                for do in range(DO):
                    ps2 = psum.tile([P, 512], F32, tag="ps",
                                    name=f"ps2_{g}_{ci}_{do}")[:, :c]
                    for k in range(KF):
                        nc.tensor.matmul(ps2, w1_ap(g, k, do),
                                         ht[:, k * c:k * c + c],
                                         start=(k == 0), stop=(k == KF - 1))
                    y_sb = ypool.tile([P, 512], BF16, tag="y",
                                      name=f"y_{g}_{ci}_{do}")[:, :c]
                    nc.vector.tensor_copy(y_sb, ps2)
                    nc.gpsimd.dma_start(yd[yi][:, do * c:(do + 1) * c], y_sb)

    nc.compile()
    return nc


def _plan(counts):
    """Choose per-core slot sizes (S1, S2) and assign expert token pieces.

    Minimizes T = S1 + S2 such that the 8 experts can be covered by 8
    pieces of size <= S1 plus 8 of size <= S2 (pieces of one expert may
    live on different cores). Falls back to one-slot-per-core (pure expert
    parallelism) if the search fails.
    """
    cmax = int(max(counts))
    order = sorted(range(E), key=lambda e: -counts[e])
    csort = [int(counts[e]) for e in order]

    def assign(S1, S2):
        from functools import lru_cache

        @lru_cache(maxsize=None)
        def feas(i, a, b):
            if i == len(csort):
                return ()
            c = csort[i]
            opts = []
            if c <= S1: opts.append((1, 0))
            if c <= S2: opts.append((0, 1))
            if c <= 2 * S2: opts.append((0, 2))
            if c <= S1 + S2: opts.append((1, 1))
            if c <= 2 * S1: opts.append((2, 0))
            if c <= S1 + 2 * S2: opts.append((1, 2))
            if c <= 2 * S1 + S2: opts.append((2, 1))
            opts.sort(key=lambda uv: (uv[0] + uv[1], S1 * uv[0] + S2 * uv[1]))
            for u, v in opts:
                if u <= a and v <= b:
                    rest = feas(i + 1, a - u, b - v)
                    if rest is not None:
                        return ((u, v),) + rest
            return None

        return feas(0, 8, 8)

    best = None
    for T in range(-(-N * 2 // E), cmax + 1):
        for S1 in range(-(-T // 2), T):
            S2 = T - S1
            sol = assign(S1, S2)
            if sol is not None:
                best = (S1, S2, sol)
                break
        if best:
            break
    if best is None:
        sizes = (cmax,)
        cores = [[(e, 0, int(counts[e]))] for e in range(E)]
        return sizes, cores

    S1, S2, sol = best
    s1_pieces, s2_pieces = [], []
    for i, (u, v) in enumerate(sol):
        e, c = order[i], csort[i]
        caps = [S1] * u + [S2] * v
        lo_ = 0
        for j, cap in enumerate(caps):
            take = min(cap, c - lo_)
            # ensure later pieces aren't left with more than they can hold
            take = max(take, c - lo_ - sum(caps[j + 1:]))
            (s1_pieces if cap == S1 else s2_pieces).append((e, lo_, take))
            lo_ += take
    while len(s1_pieces) < 8:
        s1_pieces.append((0, 0, 0))
    while len(s2_pieces) < 8:
        s2_pieces.append((0, 0, 0))
    sizes = (S1, S2)
    cores = [[s1_pieces[i], s2_pieces[i]] for i in range(8)]
    return sizes, cores


def _wflat(w_e, kt):
    """[D_in, D_out] -> [P, kt*D_out] k-major flat (kt = D_in // P)."""
    return np.ascontiguousarray(
        w_e.reshape(kt, P, -1).transpose(1, 0, 2).reshape(P, -1))


def kernel(x, routing_tensor, w0, b0, w1, b1):
    x = np.ascontiguousarray(np.asarray(x, dtype=np.float32))
    routing = np.asarray(routing_tensor, dtype=np.float32)
    w0 = np.asarray(w0, dtype=np.float32)
    b0 = np.asarray(b0, dtype=np.float32)
    w1 = np.asarray(w1, dtype=np.float32)
    b1 = np.asarray(b1, dtype=np.float32)

    idx = [np.nonzero(routing[:, e])[0] for e in range(E)]
    counts = [len(i) for i in idx]
    sizes, cores = _plan(counts)
    G = len(sizes)
    chunks = [_chunks_of(s) for s in sizes]
    flat = [(g, ci, c) for g in range(G) for ci, c in enumerate(chunks[g])]
    # token offset of chunk ci within group g
    coffs = {g: np.concatenate([[0], np.cumsum(chunks[g])]) for g in range(G)}

    nc = _cache.get(sizes)
    if nc is None:
        nc = _cache[sizes] = build_program(sizes)

    w0_bf = [_wflat(w0[e].astype(ml_dtypes.bfloat16), KD) for e in range(E)]
    w1_bf = [_wflat(w1[e].astype(ml_dtypes.bfloat16), KF) for e in range(E)]
    # reorder w0 flat into the fo-slice block layout used by the program
    w0_blk = [np.ascontiguousarray(np.concatenate(
        [wf.reshape(P, KD, F)[:, :, a:b].reshape(P, -1)
         for a, b in W0_SLICES], axis=1)) for wf in w0_bf]
    w1_blk = [np.ascontiguousarray(np.concatenate(
        [wf.reshape(P, KF, D)[:, :, a:b].reshape(P, -1)
         for a, b in W1_SLICES], axis=1)) for wf in w1_bf]
    b0_cols = [np.ascontiguousarray(b0[e, 0].reshape(KF, P).T)
               for e in range(E)]

    in_maps = []
    for core in cores:
        b0c = np.empty((P, G * KF), dtype=np.float32)
        m = {"b0": b0c}
        for g, (e, lo, cnt) in enumerate(core):
            b0c[:, g * KF:(g + 1) * KF] = b0_cols[e]
            m[f"w0_{g}"] = w0_blk[e]
            m[f"w1_{g}"] = w1_blk[e]
            for ci, c in enumerate(chunks[g]):
                o = coffs[g][ci]
                xi = np.zeros((P, KD * c), dtype=ml_dtypes.bfloat16)
                tok = idx[e][lo + o:lo + min(o + c, cnt)]
                nt = len(tok)
                if nt:
                    # x[tok].T is [D, nt]; [ko*P+p, t] -> [p][ko][t]
                    xi.reshape(P, KD, c)[:, :, :nt] = \
                        x[tok].T.astype(ml_dtypes.bfloat16) \
                        .reshape(KD, P, nt).transpose(1, 0, 2)
                m[f"x_{g}_{ci}"] = xi
        in_maps.append(m)

    res = run_bass_kernel_spmd(nc, in_maps, core_ids=list(range(8)))

    # combine: out = routing @ b1 + sum of r_e-scaled group outputs
    out = routing @ b1[:, 0, :]
    for cix, core in enumerate(cores):
        r = res.results[cix]
        for g, (e, lo, cnt) in enumerate(core):
            if cnt == 0:
                continue
            for ci, c in enumerate(chunks[g]):
                o = coffs[g][ci]
                if o >= cnt:
                    continue
                nt = min(o + c, cnt) - o
                tok = idx[e][lo + o:lo + o + nt]
                # y_{g}_{ci} is [P, DO*c] -> [p][do][t]; token t row d=do*P+p
                yc = np.asarray(r[f"y_{g}_{ci}"], dtype=np.float32) \
                    .reshape(P, DO, c)[:, :, :nt]
                yt = yc.transpose(2, 1, 0).reshape(nt, D)
                out[tok] += routing[tok, e:e + 1] * yt
    return out.astype(np.float32)
